# revision 1
# baseline (speedup 1.0000x reference)
"""Bahdanau-attention score kernel (softmax(v . tanh(W[h;enc]+b))) for 8 TRN2 cores.

Self-contained: hardcodes shapes B=32, S=2048, ENC2=600, DD=900.
Sharding: data-parallel over batch (4 batches/core), weights replicated.
"""

import contextlib
import os

import numpy as np

import concourse.bass as bass  # noqa: F401
import concourse.mybir as mybir
import concourse.tile as tile
from concourse import bacc
from concourse.bass_utils import run_bass_kernel_spmd
from concourse.masks import make_identity

F32 = mybir.dt.float32
F32R = mybir.dt.float32r
AF = mybir.ActivationFunctionType
ALU = mybir.AluOpType
AX = mybir.AxisListType

NCORES = 8
B, S, E2, DD = 32, 2048, 600, 900
IN_DIM = DD + E2            # 1500
BL = B // NCORES            # 4 batches per core
SROWS = BL * S              # 8192 s-rows per core
P = 128
TBLK = 4                    # s-tiles per block
BLK = P * TBLK              # 512
NBLK = SROWS // BLK         # 16
NCOL = SROWS // P           # 64 score columns
ECH = [(0, 128), (128, 128), (256, 128), (384, 128), (512, 88)]   # e chunks of 600
DCH = [(i * 128, 128) for i in range(7)] + [(896, 4)]             # d chunks of 900
NSP = [(0, 512), (512, 388)]                                      # N splits of 900
KA = 92          # chunk-4 contraction: 88 e-rows + 4 one-hot rows
NPRE = 3         # blocks whose transposes are emitted ahead of init

# debug bisection knobs (full kernel by default)
K_NBLK = int(os.environ.get("K_NBLK", NBLK))
K_SOFT = int(os.environ.get("K_SOFT", "1"))
K_INIT = int(os.environ.get("K_INIT", "1"))


def build():
    nc = bacc.Bacc("TRN2", target_bir_lowering=False)
    # f32r has identical bytes to f32 -- declaring inputs as f32r lets the
    # fast HWDGE DMA path (no dtype cast) feed the f32r matmuls directly
    enc_ext = nc.dram_tensor("enc", [SROWS, E2], F32R, kind="ExternalInput")
    hid_ext = nc.dram_tensor("hid", [BL, DD], F32, kind="ExternalInput")
    wt_ext = nc.dram_tensor("attn_wT", [IN_DIM, DD], F32R, kind="ExternalInput")
    b_ext = nc.dram_tensor("attn_b", [1, DD], F32, kind="ExternalInput")
    v_ext = nc.dram_tensor("v", [1, DD], F32, kind="ExternalInput")
    oh_ext = nc.dram_tensor("onehot", [BL * BL, BLK], F32R, kind="ExternalInput")
    out_ext = nc.dram_tensor("out", [BL, S], F32, kind="ExternalOutput")
    scr_dram = nc.dram_tensor("scr", [1, BL], F32)  # softmax bcast scratch

    with tile.TileContext(nc) as tc:
        with (
            tc.tile_pool(name="stat", bufs=1) as stat,
            tc.tile_pool(name="ps_t", bufs=4, space="PSUM") as ps_t,
            tc.tile_pool(name="ps_e", bufs=2, space="PSUM") as ps_e,
        ):
            # ---------------- constants ----------------
            ident_f = stat.tile([P, P], F32)
            make_identity(nc, ident_f[:, :])
            ident_r = stat.tile([P, P], F32R)
            nc.scalar.copy(ident_r[:, :], ident_f[:, :])

            enc_es = contextlib.ExitStack()
            encp = enc_es.enter_context(tc.tile_pool(name="encp", bufs=8))
            etp = enc_es.enter_context(tc.tile_pool(name="etp", bufs=4))
            initp_es = contextlib.ExitStack()
            initp = initp_es.enter_context(tc.tile_pool(name="init", bufs=1))

            # ---- DMA issue order: enc0, Wh (h_proj path), We, enc1.. ------
            enc_tiles = {}

            def issue_enc(k):
                et_ = encp.tile([P, TBLK, E2], F32R, tag="enc", name=f"enc{k}")
                nc.sync.dma_start(
                    out=et_[:, :, :],
                    in_=enc_ext.ap()[k * BLK:(k + 1) * BLK, :].rearrange(
                        "(t p) e -> p t e", p=P
                    ),
                )
                enc_tiles[k] = et_

            if K_NBLK > 0:
                issue_enc(0)

            # rhs first: it unlocks chunk0-3 matmuls for every landed block
            rhs_main = stat.tile([P, 4, DD], F32R)
            nc.sync.dma_start(
                out=rhs_main[:, :, :],
                in_=wt_ext.ap()[DD:DD + 512, :].rearrange("(c p) o -> p c o", p=P),
            )
            rhs4 = stat.tile([KA, DD], F32R)  # 88 WeT rows + 4 hb rows
            nc.sync.dma_start(out=rhs4[0:88, :], in_=wt_ext.ap()[DD + 512:IN_DIM, :])
            rhs = [rhs_main[:, c, :] for c in range(4)]

            # pre-write one-hot rows 88..91 into the chunk-4 slots; batch 0's
            # two slots must land before block 0's chunk-4 matmuls (~25us),
            # the other batches aren't read before block 4 (~70us)
            def warm_et4(b):
                for i in range(2):
                    warm = etp.tile([KA, BLK], F32R, tag=f"et4_{b}",
                                    name=f"warm{b}_{i}", bufs=2)
                    nc.sync.dma_start(
                        out=warm[88:KA, :], in_=oh_ext.ap()[b * BL:(b + 1) * BL, :]
                    )

            warm_et4(0)

            for k in range(1, min(3, K_NBLK)):
                issue_enc(k)

            whT_main = initp.tile([P, 8, DD], F32R)
            nc.sync.dma_start(
                out=whT_main[:, 0:7, :],
                in_=wt_ext.ap()[0:896, :].rearrange("(c p) o -> p c o", p=P),
            )
            nc.sync.dma_start(out=whT_main[0:4, 7, :], in_=wt_ext.ap()[896:DD, :])
            hid_stage = stat.tile([BL, DD], F32)
            nc.sync.dma_start(out=hid_stage[:, :], in_=hid_ext.ap())
            b_rep = stat.tile([BL, DD], F32)
            nc.sync.dma_start(out=b_rep[:, :], in_=b_ext.ap().partition_broadcast(BL))
            v_rep = stat.tile([P, DD], F32)
            nc.sync.dma_start(out=v_rep[:, :], in_=v_ext.ap().partition_broadcast(P))

            for k in range(3, min(6, K_NBLK)):
                issue_enc(k)
            for b in range(1, BL):
                warm_et4(b)

            scores = stat.tile([P, NCOL], F32)
            dve_scr = stat.tile([1, 4], F32)
            scT = stat.tile([NCOL, P], F32)
            e1 = stat.tile([NCOL, P], F32)
            rs = stat.tile([NCOL, 1], F32)
            absr = stat.tile([P, 2], F32)

            # DVE primes: absorb DMA sems for tiles DVE will read later
            nc.vector.tensor_copy(out=dve_scr[0:1, 0:1], in_=v_rep[0:1, 0:1])
            nc.vector.tensor_copy(out=dve_scr[0:1, 1:2], in_=b_rep[0:1, 0:1])

            # PE prime: observe ACT sem (ident_r) with one wait
            pr1 = ps_t.tile([P, BLK], F32R, tag="tp")
            nc.tensor.transpose(pr1[0:P, 0:P], ident_r[:, :], ident_r[:, :])

            # ---------------- per-block transposes + copies ----------------
            encT_blocks = {}

            def emit_transposes(k):
                bidx = k // (NBLK // BL)
                enc_t = enc_tiles[k]
                encT = []
                for c, (es, ec) in enumerate(ECH):
                    pst = ps_t.tile([P, BLK], F32R, tag="tp", name=f"pst{c}_{k}")
                    for t in range(TBLK):
                        nc.tensor.transpose(
                            pst[0:ec, t * P:(t + 1) * P],
                            enc_t[:, t, es:es + ec],
                            ident_r[:, :],
                        )
                    if c < 4:
                        et = etp.tile([ec, BLK], F32R, tag=f"et{c}",
                                      name=f"et{c}_{k}")
                    else:
                        # rows 88..91 hold the pre-written one-hot(batch)
                        et = etp.tile([KA, BLK], F32R, tag=f"et4_{bidx}",
                                      name=f"et4_{k}", bufs=2)
                    nc.scalar.copy(et[0:ec, :], pst[0:ec, :])
                    encT.append(et)
                encT_blocks[k] = encT

            if not K_INIT:
                return nc

            # software pipeline: first blocks' transposes ahead of init
            for k in range(min(NPRE, K_NBLK)):
                emit_transposes(k)

            # ---------------- init: h_proj ----------------
            # PE primes for the weight DMAs (one wait each)
            for nm, src in (("pm_w", whT_main[:, 0, 0:P]),
                            ("pm_w2", whT_main[0:4, 7, 0:P]),
                            ("pm_r", rhs_main[:, 0, 0:P]),
                            ("pm_r4", rhs4[0:88, 0:P])):
                prt = ps_t.tile([P, BLK], F32R, tag="tp", name=f"ps_{nm}")
                nc.tensor.transpose(
                    prt[0:P, 0:src.shape[0]],
                    src,
                    ident_r[0:src.shape[0], 0:src.shape[0]],
                )

            # hidden^T chunks + h_proj matmuls
            hp = ps_e.tile([BL, DD], F32, tag="ep")
            for c, (ds, dc) in enumerate(DCH):
                psh = ps_t.tile([P, BLK], F32, tag="tp")
                nc.tensor.transpose(
                    psh[0:dc, 0:BL], hid_stage[:, ds:ds + dc],
                    ident_f[0:BL, 0:BL]
                )
                hidT = initp.tile([dc, BL], F32R, tag=f"hidT{c}")
                nc.scalar.copy(hidT[:, :], psh[0:dc, 0:BL])

                for (no, nn) in NSP:
                    nc.tensor.matmul(
                        hp[:, no:no + nn], hidT[:, :],
                        whT_main[0:dc, c, no:no + nn],
                        start=(c == 0), stop=(c == len(DCH) - 1),
                    )

            # hb = h_proj + attn_b -> rhs4 rows 88..91 (f32r, SWDGE cast)
            hb_stage = initp.tile([BL, DD], F32)
            nc.vector.tensor_add(hb_stage[:, :], hp[:, :], b_rep[:, :])
            nc.gpsimd.dma_start(out=rhs4[88:KA, :], in_=hb_stage[:, :])

            # PE prime for rhs4's hb rows (SWDGE sem), 1 wait
            pr2 = ps_t.tile([P, BLK], F32R, tag="tp")
            nc.tensor.transpose(
                pr2[0:P, 0:KA], rhs4[0:KA, 0:P], ident_r[0:KA, 0:KA]
            )
            initp_es.close()

            # ---------------- main loop ----------------
            with (
                tc.tile_pool(name="zp", bufs=6) as zp,
                tc.tile_pool(name="jp", bufs=3) as jp,
            ):
                for k in range(K_NBLK):
                    bidx = k // (NBLK // BL)
                    if k not in enc_tiles:
                        issue_enc(k)
                    if k not in encT_blocks:
                        emit_transposes(k)
                    encT = encT_blocks.pop(k)

                    for t in range(TBLK):
                        eps = ps_e.tile([P, DD], F32, tag="ep")
                        for c, (es, ec) in enumerate(ECH):
                            lhs = (encT[c][:, t * P:(t + 1) * P] if c < 4
                                   else encT[4][0:KA, t * P:(t + 1) * P])
                            rr = rhs[c] if c < 4 else rhs4
                            for (no, nn) in NSP:
                                nc.tensor.matmul(
                                    eps[:, no:no + nn],
                                    lhs,
                                    rr[:, no:no + nn],
                                    start=(c == 0), stop=(c == len(ECH) - 1),
                                )
                        z = zp.tile([P, DD], F32, tag="z")
                        nc.scalar.activation(z[:, :], eps[:, :], AF.Tanh)
                        junk = jp.tile([P, DD], F32, tag="junk")
                        nc.vector.tensor_mul(junk[:, :], z[:, :], v_rep[:, :])
                        col = TBLK * k + t
                        if t == 1:
                            # one ACT reduce per block keeps ACT's DVE clock
                            # fresh (z-slot release discipline)
                            dump = jp.tile([P, DD], F32, tag="dump")
                            nc.scalar.activation(
                                dump[:, :], junk[:, :], AF.Copy,
                                accum_out=scores[:, col:col + 1],
                            )
                        else:
                            nc.vector.tensor_reduce(
                                out=scores[:, col:col + 1], in_=junk[:, :],
                                axis=AX.X, op=ALU.add,
                            )

                    # overlap softmax phase 1 with the main loop: transpose +
                    # exp each 32-column half as soon as its blocks finish
                    if K_SOFT and K_NBLK == NBLK and k in (NBLK // 2 - 1, NBLK - 1):
                        h = 0 if k == NBLK // 2 - 1 else 1
                        c0 = 32 * h
                        nc.scalar.copy(absr[:, h:h + 1],
                                       scores[:, c0 + 31:c0 + 32])
                        pss = ps_t.tile([P, BLK], F32, tag="tp", name=f"ps_sm{h}")
                        nc.tensor.transpose(pss[0:32, 0:P],
                                            scores[:, c0:c0 + 32],
                                            ident_f[:, :])
                        nc.scalar.copy(scT[c0:c0 + 32, :], pss[0:32, 0:P])
                        nc.scalar.activation(
                            e1[c0:c0 + 32, :], scT[c0:c0 + 32, :], AF.Exp,
                            accum_out=rs[c0:c0 + 32, :],
                        )
            enc_es.close()

            # ---------------- softmax phase 2 ------------------------------
            if not K_SOFT or K_NBLK < NBLK:
                return nc
            with tc.tile_pool(name="endp", bufs=1) as endp:
                ps2 = ps_t.tile([P, BLK], F32, tag="tp")
                nc.tensor.transpose(ps2[0:1, 0:NCOL], rs[:, :],
                                    ident_f[0:NCOL, 0:NCOL])
                rsT = endp.tile([1, NCOL], F32)
                nc.scalar.copy(rsT[:, :], ps2[0:1, 0:NCOL])

                rb = endp.tile([1, BL], F32)
                nc.vector.tensor_reduce(
                    out=rb[:, :],
                    in_=rsT[0:1, :].rearrange("p (b t) -> p b t", b=BL),
                    axis=AX.X, op=ALU.add,
                )
                rbi = endp.tile([1, BL], F32)
                nc.vector.reciprocal(rbi[:, :], rb[:, :])
                nc.sync.dma_start(out=scr_dram.ap(), in_=rbi[:, :])
                rfac = endp.tile([NCOL, 1], F32)
                nbt = NCOL // BL   # 16
                for bb in range(BL):
                    nc.sync.dma_start(
                        out=rfac[bb * nbt:(bb + 1) * nbt, 0:1],
                        in_=scr_dram.ap()[0:1, bb:bb + 1].partition_broadcast(nbt),
                    )
                # DVE prime on rfac
                nc.vector.tensor_copy(out=dve_scr[0:1, 2:3], in_=rfac[0:1, 0:1])
                outf = endp.tile([NCOL, P], F32)
                nc.vector.tensor_scalar_mul(outf[:, :], e1[:, :], rfac[:, 0:1])
                nc.sync.dma_start(
                    out=out_ext.ap().rearrange("b (t p) -> (b t) p", p=P),
                    in_=outf[:, :],
                )
    return nc


_CACHE = {}


def _get_nc():
    if "nc" not in _CACHE:
        nc = build()
        nc.compile()
        _CACHE["nc"] = nc
    return _CACHE["nc"]


def make_in_maps(hidden, encoder_outputs, attn_W, attn_b, v):
    in_maps = []
    for c in range(NCORES):
        bs = slice(c * BL, (c + 1) * BL)
        in_maps.append({
            "enc": np.ascontiguousarray(
                np.asarray(encoder_outputs[bs], dtype=np.float32).reshape(SROWS, E2)
            ),
            "hid": np.ascontiguousarray(np.asarray(hidden[bs], dtype=np.float32)),
            "attn_wT": np.ascontiguousarray(np.asarray(attn_W, dtype=np.float32).T),
            "attn_b": np.asarray(attn_b, dtype=np.float32).reshape(1, DD),
            "v": np.asarray(v, dtype=np.float32).reshape(1, DD),
            "onehot": np.ascontiguousarray(
                np.repeat(np.eye(BL, dtype=np.float32).reshape(BL * BL, 1),
                          BLK, axis=1)
            ),
        })
    return in_maps


def run(in_maps, trace=False, **kw):
    nc = _get_nc()
    return run_bass_kernel_spmd(nc, in_maps, core_ids=list(range(NCORES)),
                                trace=trace, **kw)


def kernel(hidden, encoder_outputs, attn_W, attn_b, v):
    in_maps = make_in_maps(hidden, encoder_outputs, attn_W, attn_b, v)
    try:
        res = run(in_maps)
    except Exception:
        # transient device states (e.g. a previously wedged core) sometimes
        # clear on retry
        res = run(in_maps)
    out = np.concatenate([res.results[c]["out"] for c in range(NCORES)], axis=0)
    return np.ascontiguousarray(out, dtype=np.float32)



# revision 9
# speedup vs baseline: 1.2168x; 1.2168x over previous
"""Bahdanau-attention score kernel (softmax(v . tanh(W[h;enc]+b))) for 8 TRN2 cores.

Self-contained: hardcodes shapes B=32, S=2048, ENC2=600, DD=900.
Sharding: data-parallel over batch (4 batches/core), weights replicated.

Design (v2):
- Host prep: enc cast to fp16 and padded to 640 cols; cols 600..603 carry a
  one-hot(batch) so the h-projection lands via 4 extra contraction rows.
  h_proj+bias (tiny: [32,900]) computed on host, appended to We^T rows.
- Device: enc chunks land TRANSPOSED in SBUF via the DMA xbar transpose
  (16-bit path), so PE runs only the 5 contraction matmuls per 128-row
  tile (N=900 single instruction, fp16 moving operand).
- ACT drains PSUM with tanh (fp16 out); DVE does a single fused
  tensor_tensor_reduce (z*v -> score column). Softmax runs per batch,
  overlapped with the main loop; only the final normalize is a tail.
"""

import os

import numpy as np

import concourse.bass as bass  # noqa: F401
import concourse.mybir as mybir
import concourse.tile as tile
from concourse import bacc
from concourse.bass_utils import run_bass_kernel_spmd
from concourse.masks import make_identity

F32 = mybir.dt.float32
F16 = mybir.dt.float16
AF = mybir.ActivationFunctionType
ALU = mybir.AluOpType
AX = mybir.AxisListType

NCORES = 8
B, S, E2, DD = 32, 2048, 600, 900
EP = 640                    # padded e dim (5 xbar chunks of 128)
BL = B // NCORES            # 4 batches per core
SROWS = BL * S              # 8192 s-rows per core
P = 128
NTIL = SROWS // P           # 64 score tiles/columns
TPB = S // P                # 16 tiles per batch
NCH = 5                     # e chunks of 128 (last: 88 enc + 4 one-hot)
K4 = 92                     # chunk-4 contraction rows
# xbar transpose segments: the ucode instruction caps at 512 source rows
# and corrupts the tail at exactly 512, so use 496-row segments
SEGS = [(0, 496), (496, 496), (992, 496), (1488, 496), (1984, 64)]

K_TILES = int(os.environ.get("K_TILES", NTIL))


def build():
    nc = bacc.Bacc("TRN2", target_bir_lowering=False)
    enc_ext = nc.dram_tensor("enc", [SROWS, EP], F16, kind="ExternalInput")
    wcat_ext = nc.dram_tensor("wcat", [512 + K4, DD], F16, kind="ExternalInput")
    v_ext = nc.dram_tensor("v", [1, DD], F16, kind="ExternalInput")
    out_ext = nc.dram_tensor("out", [BL, S], F32, kind="ExternalOutput")
    scr_dram = nc.dram_tensor("scr", [1, BL], F32)  # softmax bcast scratch

    with tile.TileContext(nc) as tc:
        with (
            tc.tile_pool(name="stat", bufs=1) as stat,
            tc.tile_pool(name="encp", bufs=2) as encp,
            tc.tile_pool(name="zp", bufs=3) as zp,
            tc.tile_pool(name="jp", bufs=2) as jp,
            tc.tile_pool(name="ps_e", bufs=3, space="PSUM") as ps_e,
            tc.tile_pool(name="ps_t", bufs=2, space="PSUM") as ps_t,
        ):
            # ---------------- weights + constants ----------------
            rhs_main = stat.tile([P, 4, DD], F16)
            for h in range(4):
                nc.sync.dma_start(
                    out=rhs_main[:, h, :],
                    in_=wcat_ext.ap()[h * P:(h + 1) * P, :],
                )
            rhs4 = stat.tile([K4, DD], F16)
            nc.sync.dma_start(out=rhs4[:, :], in_=wcat_ext.ap()[512:512 + K4, :])

            # ---- enc xbar-transpose DMAs: batch 0 first ----
            enc_tiles = {}

            def issue_enc(b):
                for c in range(NCH):
                    et = encp.tile([P, S], F16, tag=f"enc{c}", name=f"enc{b}_{c}")
                    for (g0, gn) in SEGS:
                        nc.sync.dma_start(
                            out=et[:, g0:g0 + gn],
                            in_=enc_ext.ap()[
                                b * S + g0:b * S + g0 + gn,
                                c * P:(c + 1) * P,
                            ],
                            transpose=True,
                        )
                    enc_tiles[(b, c)] = et

            issue_enc(0)

            v_rep = stat.tile([P, DD], F16)
            nc.sync.dma_start(out=v_rep[:, :], in_=v_ext.ap().partition_broadcast(P))

            ident_f = stat.tile([P, P], F32)
            make_identity(nc, ident_f[:, :])

            for b in range(1, BL):
                issue_enc(b)

            scores = stat.tile([P, NTIL], F32)
            e1 = stat.tile([TPB, BL, P], F32)
            rs = stat.tile([TPB, BL], F32)
            rb = stat.tile([BL, 1], F32)
            rbi = stat.tile([BL, 1], F32)
            rfac = stat.tile([TPB, BL], F32)
            outf = stat.tile([TPB, BL, P], F32)

            # ---------------- main loop ----------------
            for t in range(K_TILES):
                b, ti = divmod(t, TPB)
                eps = ps_e.tile([P, DD], F32, tag="ep")
                for c in range(NCH):
                    et = enc_tiles[(b, c)]
                    kk = P if c < 4 else K4
                    rr = rhs_main[:, c, :] if c < 4 else rhs4[:, :]
                    for (no, nn) in ((0, 512), (512, 388)):
                        nc.tensor.matmul(
                            eps[:, no:no + nn],
                            et[0:kk, ti * P:(ti + 1) * P],
                            rr[:, no:no + nn],
                            start=(c == 0), stop=(c == NCH - 1),
                        )
                z = zp.tile([P, DD], F16, tag="z")
                nc.scalar.activation(z[:, :], eps[:, :], AF.Tanh)
                junk = jp.tile([P, DD], F16, tag="junk")
                nc.vector.tensor_mul(junk[:, :], z[:, :], v_rep[:, :])
                nc.vector.tensor_reduce(
                    out=scores[:, t:t + 1], in_=junk[:, :],
                    axis=AX.X, op=ALU.add,
                )

                # per-batch softmax phase 1, overlapped with the main loop
                if ti == TPB - 1 and K_TILES == NTIL:
                    c0 = b * TPB
                    pst = ps_t.tile([P, P], F32, tag="tp")
                    nc.tensor.transpose(
                        pst[0:TPB, :], scores[:, c0:c0 + TPB], ident_f[:, :]
                    )
                    nc.scalar.activation(
                        e1[:, b, :], pst[0:TPB, :], AF.Exp,
                        accum_out=rs[:, b:b + 1],
                    )

            if K_TILES < NTIL:
                return nc

            # ---------------- softmax tail ----------------
            # rs [TPB, BL]: per-batch partial sums; cross-partition sum via PE
            ps2 = ps_t.tile([P, P], F32, tag="tp")
            nc.tensor.transpose(ps2[0:BL, 0:TPB], rs[:, :], ident_f[0:TPB, 0:TPB])
            nc.vector.tensor_reduce(
                out=rb[:, :], in_=ps2[0:BL, 0:TPB], axis=AX.X, op=ALU.add,
            )
            nc.vector.reciprocal(rbi[:, :], rb[:, :])
            nc.sync.dma_start(
                out=scr_dram.ap().rearrange("a b -> b a"), in_=rbi[:, :]
            )
            nc.sync.dma_start(
                out=rfac[:, :], in_=scr_dram.ap().partition_broadcast(TPB)
            )
            for bb in range(BL):
                nc.vector.tensor_scalar_mul(
                    outf[:, bb, :], e1[:, bb, :], rfac[:, bb:bb + 1]
                )
            nc.sync.dma_start(
                out=out_ext.ap().rearrange("b (t p) -> t b p", p=P),
                in_=outf[:, :, :],
            )
    return nc


_CACHE = {}


def _get_nc():
    if "nc" not in _CACHE:
        nc = build()
        nc.compile()
        _CACHE["nc"] = nc
    return _CACHE["nc"]


def make_in_maps(hidden, encoder_outputs, attn_W, attn_b, v):
    hidden = np.asarray(hidden, dtype=np.float32)
    encoder_outputs = np.asarray(encoder_outputs, dtype=np.float32)
    attn_W = np.asarray(attn_W, dtype=np.float32)
    attn_b = np.asarray(attn_b, dtype=np.float32)
    v = np.asarray(v, dtype=np.float32)

    WeT = np.ascontiguousarray(attn_W[:, DD:].T)          # [600, 900]
    hb_all = hidden @ attn_W[:, :DD].T + attn_b           # [32, 900]
    v16 = v.astype(np.float16).reshape(1, DD)

    in_maps = []
    for c in range(NCORES):
        bs = slice(c * BL, (c + 1) * BL)
        encp = np.zeros((SROWS, EP), dtype=np.float16)
        encp[:, :E2] = encoder_outputs[bs].reshape(SROWS, E2)
        for b in range(BL):
            encp[b * S:(b + 1) * S, E2 + b] = 1.0
        wcat = np.concatenate([WeT, hb_all[bs]], axis=0).astype(np.float16)
        in_maps.append({
            "enc": encp,
            "wcat": np.ascontiguousarray(wcat),
            "v": v16,
        })
    return in_maps


def run(in_maps, trace=False, **kw):
    nc = _get_nc()
    return run_bass_kernel_spmd(nc, in_maps, core_ids=list(range(NCORES)),
                                trace=trace, **kw)


def kernel(hidden, encoder_outputs, attn_W, attn_b, v):
    in_maps = make_in_maps(hidden, encoder_outputs, attn_W, attn_b, v)
    try:
        res = run(in_maps)
    except Exception:
        # transient device states (e.g. a previously wedged core) sometimes
        # clear on retry
        res = run(in_maps)
    out = np.concatenate([res.results[c]["out"] for c in range(NCORES)], axis=0)
    return np.ascontiguousarray(out, dtype=np.float32)
